# revision 11
# baseline (speedup 1.0000x reference)
"""LNN / echo-state step on 8 TRN2 NeuronCores.

Computes state = 0.7*prev_state + 0.3*tanh(inputs @ Wi^T + prev_state @ Wr^T)
for B=8192, IN=2048, R=4096 (fp32 in/out).

Strategy: data-parallel over batch. Each of the 8 cores gets a 1024-row batch
shard and the full (replicated) weights, computes its shard's output with no
collectives, and the host reassembles.

Per-core kernel (fp8e4m3 DoubleRow matmuls — 2 MACs/cell/cycle, ~1.7x over
fp32r/bf16 streaming):
  - Host quantizes activations (x, h) and weights to fp8 e4m3. Weights are
    pre-scaled by 64 so their std (~0.02) lands in e4m3's normal range;
    the 1/64 is folded into the tanh via the scalar engine's scale arg.
  - out^T[r, b] accumulates over 24 DoubleRow pairs (256 contraction rows
    each): pairs 0-7 contract x^T against Wi^T, pairs 8-23 contract h^T
    against Wr^T. fp8 activations (6 MB) stay resident in SBUF; fp8 weight
    pair-tiles stream from HBM per output m-tile.
  - epilogue per [128, 512] tile: tanh(psum/64) on ScalarE, then
    out = 0.7*h_fp32 + 0.3*tanh on VectorE (h^T fp32 tiles streamed from
    HBM per m-tile), DMA back to HBM.

Host-side numpy does the transposes/tiling/quantization so every DMA is
contiguous.
"""

import numpy as np
import ml_dtypes

import concourse.bass as bass
import concourse.mybir as mybir
from concourse import bacc
from concourse.tile import TileContext

P = 128
B_FULL, IN_DIM, R_DIM = 8192, 2048, 4096
N_CORES = 8
B_SHARD = B_FULL // N_CORES
LEAK = 0.3
W_SCALE = 64.0
USE_SW = True  # DoubleRowSwInterleave (contiguous weight loads) vs DoubleRow

F8NP = ml_dtypes.float8_e4m3


import contextlib


@contextlib.contextmanager
def _null_ctx():
    yield None


def build_program(in_dim=IN_DIM, r_dim=R_DIM, b_shard=B_SHARD, kpc=6, n_tile=512,
                  t_loop=None):
    """Emit the per-core Bass program. Returns (nc, meta).

    t_loop: if set, wrap the whole body in a hardware For_i loop that runs it
    t_loop times back-to-back on device (timing use only — slope over t_loop
    cancels dispatch overhead exactly)."""
    kp_x = in_dim // (2 * P)    # DoubleRow pairs from the input matmul
    kp_h = r_dim // (2 * P)     # DoubleRow pairs from the reservoir matmul
    kp = kp_x + kp_h            # total fused contraction pairs
    mt = r_dim // P             # output row tiles (R on partitions)
    nt = b_shard // n_tile      # output column tiles
    nchunk = kp // kpc          # weight DMA chunks per m-tile
    assert kp % kpc == 0 and b_shard % n_tile == 0

    f32 = mybir.dt.float32
    f8 = mybir.dt.float8e4
    Tanh = mybir.ActivationFunctionType.Tanh
    DR = (mybir.MatmulPerfMode.DoubleRowSwInterleave if USE_SW
          else mybir.MatmulPerfMode.DoubleRow)

    nc = bacc.Bacc("TRN2", target_bir_lowering=False, debug=False)

    acts_d = nc.dram_tensor("acts", [kp, P, 2 * b_shard], f8, kind="ExternalInput")
    wts_d = nc.dram_tensor("wts", [mt, nchunk, P, kpc * 2 * P], f8, kind="ExternalInput")
    h32_d = nc.dram_tensor("h32", [mt, P, b_shard], f32, kind="ExternalInput")
    out_d = nc.dram_tensor("out", [mt, P, b_shard], f32, kind="ExternalOutput")

    with TileContext(nc) as tc:
        with (
            tc.tile_pool(name="act_pool", bufs=kp) as apool,
            tc.tile_pool(name="w_pool", bufs=3) as wpool,
            tc.tile_pool(name="h_pool", bufs=3) as hpool,
            tc.tile_pool(name="t_pool", bufs=2) as tpool,
            tc.tile_pool(name="o_pool", bufs=2) as opool,
            tc.tile_pool(name="ps_pool", bufs=4, space="PSUM") as pspool,
            (tc.For_i(0, t_loop) if t_loop is not None else _null_ctx()),
        ):
            act_tiles = []
            for k in range(kp):
                at = apool.tile([P, 2, b_shard], f8, tag="act", name=f"act{k}")
                nc.sync.dma_start(at[:], acts_d[k])
                act_tiles.append(at)

            for m in range(mt):
                ht = hpool.tile([P, b_shard], f32, tag="h")
                nc.sync.dma_start(ht[:], h32_d[m])
                psums = [pspool.tile([P, n_tile], f32, tag="ps", name=f"ps{m}_{n}")
                         for n in range(nt)]
                for ch in range(nchunk):
                    wshape = [P, kpc, 2 * P] if USE_SW else [P, kpc, 2, P]
                    wc = wpool.tile(wshape, f8, tag="w")
                    nc.sync.dma_start(wc[:], wts_d[m, ch])
                    for kl in range(kpc):
                        k = ch * kpc + kl
                        lhsT = wc[:, kl]
                        for n in range(nt):
                            rhs = act_tiles[k][:, :, n * n_tile:(n + 1) * n_tile]
                            nc.tensor.matmul(
                                psums[n][:],
                                lhsT,
                                rhs,
                                start=(k == 0),
                                stop=(k == kp - 1),
                                perf_mode=DR,
                            )
                for n in range(nt):
                    t = tpool.tile([P, n_tile], f32, tag="t")
                    nc.scalar.activation(t[:], psums[n][:], Tanh, scale=1.0 / W_SCALE)
                    o = opool.tile([P, n_tile], f32, tag="o")
                    h_slice = ht[:, n * n_tile:(n + 1) * n_tile]
                    nc.vector.tensor_scalar_mul(o[:], h_slice, 1.0 - LEAK)
                    nc.vector.scalar_tensor_tensor(
                        o[:], t[:], LEAK, o[:],
                        mybir.AluOpType.mult, mybir.AluOpType.add,
                    )
                    nc.sync.dma_start(out_d[m, :, n * n_tile:(n + 1) * n_tile], o[:])

    nc.compile()
    meta = dict(in_dim=in_dim, r_dim=r_dim, b_shard=b_shard, kpc=kpc,
                n_tile=n_tile, kp_x=kp_x, kp_h=kp_h, kp=kp, mt=mt, nt=nt,
                nchunk=nchunk)
    return nc, meta


def pack_weights(input_weights, reservoir_weights, kpc=6):
    """[R, IN] + [R, R] fp32 -> [mt, nchunk, P, kpc*2*P] fp8, tiled for
    contiguous DMA and DoubleRow pair layout [P, 2, P] per (m, pair)."""
    w = np.concatenate(
        [np.ascontiguousarray(input_weights.T), np.ascontiguousarray(reservoir_weights.T)],
        axis=0,
    )  # [in+r, r]: w[k, r]
    w8 = (w * W_SCALE).astype(F8NP)
    k_dim, r_dim = w8.shape
    kp, mt = k_dim // (2 * P), r_dim // P
    nchunk = kp // kpc
    # w8[k, r]: k = 256*pair + 128*j + p, r = 128*m + c -> [m, pair, p, j, c]
    t = w8.reshape(kp, 2, P, mt, P).transpose(3, 0, 2, 1, 4)  # [mt, kp, P, 2, P]
    if USE_SW:
        # SwInterleave layout per partition: A127 B127 A126 B126 ... A0 B0
        # (planes interleaved per column, columns reversed)
        t = np.ascontiguousarray(t[..., ::-1].transpose(0, 1, 2, 4, 3))
    t = t.reshape(mt, nchunk, kpc, P, 2, P).transpose(0, 1, 3, 2, 4, 5)
    return np.ascontiguousarray(t.reshape(mt, nchunk, P, kpc * 2 * P))


def pack_acts(x_shard, h_shard):
    """[b, in] + [b, r] fp32 -> [kp, P, 2*b] fp8 (transposed, DoubleRow
    pair-tiled: plane j at partition p covers k = 256*pair + 128*j + p)."""
    a = np.concatenate([x_shard.T, h_shard.T], axis=0)  # [in+r, b]
    a8 = a.astype(F8NP)
    k_dim, b = a8.shape
    return np.ascontiguousarray(a8.reshape(k_dim // (2 * P), 2 * P, b)
                                .reshape(-1, 2, P, b).transpose(0, 2, 1, 3)
                                .reshape(-1, P, 2 * b))


def pack_h32(h_shard):
    """[b, r] fp32 -> [mt, P, b] transposed tiles for the epilogue."""
    b, r = h_shard.shape
    return np.ascontiguousarray(h_shard.T.reshape(r // P, P, b))


_CACHE = {}


def make_in_maps(inputs, prev_state, input_weights, reservoir_weights):
    x = np.ascontiguousarray(np.asarray(inputs, dtype=np.float32))
    h = np.ascontiguousarray(np.asarray(prev_state, dtype=np.float32))
    wi = np.asarray(input_weights, dtype=np.float32)
    wr = np.asarray(reservoir_weights, dtype=np.float32)
    assert x.shape == (B_FULL, IN_DIM) and h.shape == (B_FULL, R_DIM)

    wts = pack_weights(wi, wr)
    in_maps = []
    for c in range(N_CORES):
        sl = slice(c * B_SHARD, (c + 1) * B_SHARD)
        in_maps.append({
            "acts": pack_acts(x[sl], h[sl]),
            "wts": wts,
            "h32": pack_h32(h[sl]),
        })
    return in_maps


def kernel(inputs, prev_state, input_weights, reservoir_weights):
    from concourse import bass_utils

    if "nc" not in _CACHE:
        _CACHE["nc"], _CACHE["meta"] = build_program()
    nc = _CACHE["nc"]

    in_maps = make_in_maps(inputs, prev_state, input_weights, reservoir_weights)
    res = bass_utils.run_bass_kernel_spmd(nc, in_maps, core_ids=list(range(N_CORES)))

    out = np.empty((B_FULL, R_DIM), dtype=np.float32)
    for c in range(N_CORES):
        o = res.results[c]["out"]  # [mt, P, b_shard]
        out[c * B_SHARD:(c + 1) * B_SHARD] = o.reshape(R_DIM, B_SHARD).T
    return out
